# revision 34
# baseline (speedup 1.0000x reference)
"""MatAnyone memory-readout kernel for 8 Trainium2 NeuronCores (fp8 DoubleRow).

Math (per batch b, query pixel n, memory slot t):
  sim[t,n] = ms[t]*(-a_sq + 2ab - b_sq)[t,n]/sqrt(CK)
  aff      = softmax_t(sim);  R[c,n] = sum_t mv[c,t]*aff[t,n]
  out[c,n] = R[c,n]*p[n] + lv[c,n]*(1-p[n])

Sharding: 8 cores = 2 batches x 4 query-pixel shards (n = 576 per core).

Per-core plan (fp8 e4m3 DoubleRow matmuls; DR = K-paired contraction,
out = sum_i W[:,i].T @ X[:,i], both operands fp8, ~2x column rate):
  sim: Ki=65 padded to 128. lhsT pairs = [ms*mk^2/2 | 2*ms*mk] per channel
       plus a 65th row (2ms | ms/4) that folds in -b_sq*ms/8 (fp8 residual
       correction in the second slot). rhs pairs = [-qe/2 | qe*qk/4] plus
       (-b_sq/8 | 8*residual). One DR matmul per (t-tile, n-half) writes
       psum = 2*sim; halves live at 512-aligned offsets (psum bank-crossing
       matmul writes are broken on trn2 - everything is bank-aligned).
       ACT: E = exp(0.5*psum) -> fp8, one instr per t-tile via 3D AP.
  R:   t-tiles paired (K=256 over t): lhsT = mv pair-chunk [128,2,128],
       rhs = E-pair [128,2,288]. 4 DR matmuls per pair, accumulated over
       all 72 pairs in psum.
  Z:   odd pairs + the last few on PE (DR ones-weight, M=1, N=144 segments
       in the R banks' gap columns); remaining pairs on DVE (G2 += E-pair);
       G2's partition sums are folded into the same psum Z segments by bf16
       ones-matmuls overlapped with the loop tail. 1/(2Z) runs on ACT
       (Reciprocal, accuracy is ample for the 2e-2 gate), with 2x folded
       into p on the host.
  PSUM (8 banks): RZ tile [128,2048] = R quarters @ bank starts (cols
       (2k+hh)*512) + Z segments @ s*512+288 (4 banks); sim pool
       [128,1024] x 2 bufs (2 banks each). PE warmup matmuls trip the HAM
       clock-gate to 8/8 during the initial DMA wait.
  out = R*(2p/2Z) + lv*(1-p) in bf16; lv*(1-p) precomputed on host.
"""

import sys

for _p in ("/opt/trn_rl_repo", "/root/.axon_site/_ro/trn_rl_repo"):
    if _p not in sys.path:
        sys.path.insert(0, _p)

from contextlib import ExitStack

import numpy as np
import ml_dtypes

import concourse.bass as bass
from concourse import mybir
from concourse.bacc import Bacc
from concourse.tile import TileContext
from concourse.bass_utils import run_bass_kernel_spmd

F32 = mybir.dt.float32
BF16 = mybir.dt.bfloat16
FP8 = mybir.dt.float8e4
EXP = mybir.ActivationFunctionType.Exp
DR = mybir.MatmulPerfMode.DoubleRow
E4M3 = ml_dtypes.float8_e4m3

B, CK, CV, T, H, W = 2, 64, 256, 8, 48, 48
HW = H * W            # 2304
THW = T * HW          # 18432
NCORE = HW // 4       # 576 query pixels per core
NH = NCORE // 2       # 288 per n-half
TT = THW // 128       # 144 t-tiles
NPAIR = TT // 2       # 72 t-tile pairs
SKEW = 3              # pairs of lag between exp and readout
CHP = 8               # pairs per streamed mkw chunk
Z_PE_MOD = 10**9          # pair a accumulates Z on PE iff a % Z_PE_MOD == 0

_CACHE = {}


def _fp8(x):
    return np.clip(x, -240.0, 240.0).astype(E4M3)


def build_program():
    nc = Bacc(name="matanyone_fp8dr")

    qw_h = nc.declare_dram_parameter("qw", [128, 2 * NCORE], FP8, isOutput=False)
    mkw_h = nc.declare_dram_parameter("mkw", [128, TT * 256], FP8,
                                      isOutput=False)
    mvw_h = nc.declare_dram_parameter("mvw", [128, NPAIR * 512], FP8,
                                      isOutput=False)
    lvw2_h = nc.declare_dram_parameter("lvw2", [CV, NCORE], BF16,
                                       isOutput=False)
    p_h = nc.declare_dram_parameter("p", [1, NCORE], F32, isOutput=False)
    cz_h = nc.declare_dram_parameter("c_onesz", [128, 32], FP8, isOutput=False)
    cw_h = nc.declare_dram_parameter("c_warm", [128, 288], FP8, isOutput=False)
    cb_h = nc.declare_dram_parameter("c_onesb", [128, 1], BF16, isOutput=False)
    cb1_h = nc.declare_dram_parameter("c_onesb1", [1, 128], BF16, isOutput=False)
    out_h = nc.declare_dram_parameter("out", [CV, NCORE], BF16, isOutput=True)


    with TileContext(nc) as tc, ExitStack() as ctx:
        persist = ctx.enter_context(tc.tile_pool(name="persist", bufs=1))
        ps_rz0 = ctx.enter_context(tc.tile_pool(name="psrz", bufs=1,
                                                space="PSUM"))
        mvpool = ctx.enter_context(tc.tile_pool(name="mv", bufs=1))
        m2pool = ctx.enter_context(tc.tile_pool(name="m2", bufs=2))
        epool = ctx.enter_context(tc.tile_pool(name="E", bufs=SKEW + 2))
        ps_sim = ctx.enter_context(tc.tile_pool(name="pssim", bufs=2,
                                                space="PSUM"))
        fin = ctx.enter_context(tc.tile_pool(name="fin", bufs=1))

        # RZ: R quarters (k,hh) @ (2k+hh)*512 + Z segs @ s*512+288
        rz = ps_rz0.tile([128, 2048], F32, tag="rz")

        # ---- persistent inputs / constants --------------------------------
        onesz = persist.tile([128, 32], FP8, tag="onesz")
        nc.sync.dma_start(out=onesz[:], in_=cz_h[:])
        wrm = persist.tile([128, 288], FP8, tag="wrm")
        nc.sync.dma_start(out=wrm[:], in_=cw_h[:])
        qw = persist.tile([128, 2 * NCORE], FP8, tag="qw")
        for _o in (0, 576):   # halves used by hh=0 first
            nc.sync.dma_start(out=qw[:, _o:_o + NH], in_=qw_h[:, _o:_o + NH])
        for _o in (288, 864):
            nc.sync.dma_start(out=qw[:, _o:_o + NH], in_=qw_h[:, _o:_o + NH])
        onesb = persist.tile([128, 1], BF16, tag="onesb")
        nc.sync.dma_start(out=onesb[:], in_=cb_h[:])
        onesb1 = persist.tile([1, 128], BF16, tag="onesb1")
        nc.sync.dma_start(out=onesb1[:], in_=cb1_h[:])
        g2 = persist.tile([128, 2 * NCORE], F32, tag="g2")

        qw3 = qw.rearrange("p (i n) -> p i n", i=2)
        onesz3 = onesz.rearrange("p (i m) -> p i m", i=2)

        # PE warmup: real-sized matmuls into the Z-gap region trip the HAM
        # to 8/8 during the initial DMA wait; the first real Z matmul
        # (start=True) resets the region.
        wrm3 = wrm.rearrange("p (i n) -> p i n", i=2)
        for _w in range(28):
            nc.tensor.matmul(rz[0:1, 288:432], onesz3[:, :, 0:1], wrm3[:],
                             start=True, stop=True, perf_mode=DR)

        # resident mv weights, DMA'd in chunks inside the loop
        mvw = mvpool.tile([128, NPAIR * 512], FP8, tag="mvw")

        e_tiles = {}
        mkc = None

        def pair_front(a):
            nonlocal mkc
            if a % CHP == 0:
                g = a // CHP
                mkc = m2pool.tile([128, CHP * 512], FP8, tag="mkc")
                if g == 0:
                    for q in range(CHP):
                        nc.sync.dma_start(
                            out=mkc[:, q * 512:(q + 1) * 512],
                            in_=mkw_h[:, q * 512:(q + 1) * 512])
                    for q in range(CHP):
                        nc.scalar.dma_start(
                            out=mvw[:, q * 512:(q + 1) * 512],
                            in_=mvw_h[:, q * 512:(q + 1) * 512])
                else:
                    nc.sync.dma_start(
                        out=mkc[:],
                        in_=mkw_h[:, g * CHP * 512:(g + 1) * CHP * 512])
                    nc.sync.dma_start(
                        out=mvw[:, g * CHP * 512:(g + 1) * CHP * 512],
                        in_=mvw_h[:, g * CHP * 512:(g + 1) * CHP * 512])
            e = epool.tile([128, 2 * NCORE], FP8, tag="E")

            def sim_tile(j):
                wsl = mkc[:, (a % CHP) * 512 + j * 256:(a % CHP) * 512
                          + (j + 1) * 256]
                w3 = wsl.rearrange("p (i m) -> p i m", i=2)
                sim = ps_sim.tile([128, 1024], F32, tag="sim")
                for hh in (0, 1):
                    nc.tensor.matmul(sim[:, hh * 512:hh * 512 + NH], w3,
                                     qw3[:, :, hh * NH:(hh + 1) * NH],
                                     start=True, stop=True, perf_mode=DR)
                sim3 = sim.rearrange("p (i n) -> p i n", i=2)[:, :, 0:NH]
                e2 = e[:, j * NCORE:(j + 1) * NCORE].rearrange(
                    "p (i n) -> p i n", i=2)
                nc.scalar.activation(e2, sim3, EXP, scale=0.5)

            e_tiles[a] = (e, sim_tile)

        def pair_back_k(a, k):
            e = e_tiles[a][0]
            e3 = e.rearrange("p (i n) -> p i n", i=2)
            st, sp = (a == 0), (a == NPAIR - 1)
            wsl = mvw[:, a * 512 + k * 256:a * 512 + (k + 1) * 256]
            w3 = wsl.rearrange("p (i m) -> p i m", i=2)
            for hh in (0, 1):
                q = (2 * k + hh) * 512
                nc.tensor.matmul(
                    rz[:, q:q + NH],
                    w3, e3[:, :, hh * NH:(hh + 1) * NH],
                    start=st, stop=sp, perf_mode=DR)

        def pair_back_z(a):
            e = e_tiles.pop(a)[0]
            e3 = e.rearrange("p (i n) -> p i n", i=2)
            if a % 2 == 1 or a >= NPAIR - 5:
                for s in range(4):
                    nc.tensor.matmul(
                        rz[0:1, s * 512 + 288:s * 512 + 432],
                        onesz3[:, :, 0:1], e3[:, :, s * 144:(s + 1) * 144],
                        start=(a == 1), stop=False, perf_mode=DR)
            elif a == 0:
                nc.vector.tensor_copy(g2[:], e[:])
            else:
                nc.vector.tensor_add(g2[:], g2[:], e[:])
            if a == NPAIR - 5:
                # all remaining Z goes to PE; G2 is final -> fold it now so
                # the cast+fold matmuls overlap the tail of the main loop
                gb = fin.tile([128, 2 * NCORE], BF16, tag="gb")
                nc.vector.tensor_copy(gb[:], g2[:])
                for s in range(4):
                    for i in (0, 1):
                        nc.tensor.matmul(
                            rz[0:1, s * 512 + 288:s * 512 + 432], onesb[:],
                            gb[:, i * NCORE + s * 144:i * NCORE
                               + (s + 1) * 144],
                            start=False, stop=(i == 1))

        for a in range(NPAIR + SKEW):
            if a < NPAIR:
                pair_front(a)
                e_tiles[a][1](0)
                e_tiles[a][1](1)
            if a >= SKEW:
                pair_back_k(a - SKEW, 0)
                pair_back_k(a - SKEW, 1)
                pair_back_z(a - SKEW)

        # ---- finalize ------------------------------------------------------
        p_sb = persist.tile([1, NCORE], F32, tag="p")
        nc.sync.dma_start(out=p_sb[:], in_=p_h[:])
        lvw2 = []
        for k in (0, 1):
            t = persist.tile([128, NCORE], BF16, tag=f"lvw2{k}")
            nc.sync.dma_start(out=t[:], in_=lvw2_h[k * 128:(k + 1) * 128, :])
            lvw2.append(t)
        # 1/(2Z) on the scalar engine straight from psum (p has 2x folded in)
        rzv = fin.tile([1, NCORE], F32, tag="rzv")
        rz4 = rz.rearrange("p (s c) -> p s c", c=512)[0:1, :, 288:432]
        eng = nc.scalar
        eng.add_instruction(mybir.InstActivation(
            name=nc.get_next_instruction_name(),
            func=mybir.ActivationFunctionType.Reciprocal,
            ins=[eng.lower_ap(rz4),
                 mybir.ImmediateValue(dtype=mybir.dt.float32, value=0.0),
                 mybir.ImmediateValue(dtype=mybir.dt.float32, value=2.0),
                 mybir.ImmediateValue(dtype=mybir.dt.float32, value=0.0)],
            outs=[eng.lower_ap(rzv.rearrange("p (s n) -> p s n", s=4))]))
        w1 = fin.tile([1, NCORE], BF16, tag="w1")
        nc.vector.tensor_mul(w1[:], rzv[:], p_sb[:])          # 2p / 2Z

        w1s = fin.tile([128, NCORE], F32, tag="w1s")
        wt = ps_sim.tile([128, 1024], F32, tag="sim")
        for hh in (0, 1):
            nc.tensor.matmul(wt[:, hh * 512:hh * 512 + NH], onesb1[:],
                             w1[:, hh * NH:(hh + 1) * NH],
                             start=True, stop=True)
        wt3 = wt.rearrange("p (i n) -> p i n", i=2)[:, :, 0:NH]
        nc.vector.tensor_copy(w1s.rearrange("p (i n) -> p i n", i=2), wt3)

        for k in (0, 1):
            o = fin.tile([128, NCORE], BF16, tag="o", bufs=2)
            rk = rz[:, k * 1024:k * 1024 + 1024].rearrange(
                "p (i n) -> p i n", i=2)[:, :, 0:NH]
            nc.vector.tensor_mul(o.rearrange("p (i n) -> p i n", i=2), rk,
                                 w1s.rearrange("p (i n) -> p i n", i=2))
            nc.vector.tensor_add(o[:], o[:], lvw2[k][:])
            nc.sync.dma_start(out=out_h[k * 128:(k + 1) * 128, 0:NH],
                              in_=o[:, 0:NH])
            nc.scalar.dma_start(out=out_h[k * 128:(k + 1) * 128, NH:NCORE],
                                in_=o[:, NH:NCORE])

    nc.finalize()
    return nc


def _get_program():
    if "nc" not in _CACHE:
        _CACHE["nc"] = build_program()
    return _CACHE["nc"]


def _make_in_maps(query_key, query_selection, memory_key, memory_shrinkage,
                  msk_value, uncert_prob):
    qk = np.asarray(query_key, np.float32).reshape(B, CK, HW)
    qe = np.asarray(query_selection, np.float32).reshape(B, CK, HW)
    mk = np.asarray(memory_key, np.float32).reshape(B, CK, THW)
    ms = np.asarray(memory_shrinkage, np.float32).reshape(B, THW)
    mv = np.asarray(msk_value, np.float32).reshape(B, CV, THW)
    lv = np.asarray(msk_value, np.float32).reshape(B, CV, T, HW)[:, :, T - 1, :]
    p = np.asarray(uncert_prob, np.float32).reshape(B, HW)

    # per-batch sim weights: [65, TT, 2, 128]
    mkw_b = []
    mvw_b = []
    for b in range(B):
        mk3 = mk[b].reshape(CK, TT, 128)               # [c, tau, m]
        ms3 = ms[b].reshape(TT, 128)                   # [tau, m]
        mkw = np.zeros((128, TT, 2, 128), np.float32)
        mkw[:CK, :, 0, :] = ms3[None] * mk3 * mk3 * 0.5
        mkw[:CK, :, 1, :] = 2.0 * ms3[None] * mk3
        mkw[64, :, 0, :] = 2.0 * ms3
        mkw[64, :, 1, :] = 0.25 * ms3
        mkw_b.append(_fp8(mkw).reshape(128, TT * 256))
        # mv DR weights: [p, a, k, i, m] = mv[k*128+m, (2a+i)*128+p]
        tmp = mv[b].reshape(2, 128, NPAIR, 2, 128)     # [k, m, a, i, p]
        mvw = tmp.transpose(4, 2, 0, 3, 1).reshape(128, NPAIR * 512)
        mvw_b.append(_fp8(mvw))

    in_maps = []
    for core in range(8):
        b, s = divmod(core, 4)
        sl = slice(s * NCORE, (s + 1) * NCORE)
        qks, qes = qk[b, :, sl], qe[b, :, sl]
        bsq = np.einsum("cn,cn->n", qes, qks * qks)    # [576]
        qwf = np.zeros((128, 2, NCORE), np.float32)
        qwf[:CK, 0] = -0.5 * qes
        qwf[:CK, 1] = 0.25 * qes * qks
        b0 = _fp8(-bsq / 8.0)
        qwf[64, 0] = b0.astype(np.float32)
        qwf[64, 1] = 8.0 * (-bsq / 8.0 - b0.astype(np.float32))
        ps = p[b, sl]
        in_maps.append({
            "qw": _fp8(qwf).reshape(128, 2 * NCORE),
            "mkw": mkw_b[b],
            "mvw": mvw_b[b],
            "lvw2": np.ascontiguousarray(
                lv[b, :, sl] * (1.0 - ps)[None, :]).astype(ml_dtypes.bfloat16),
            "p": np.ascontiguousarray(ps).reshape(1, NCORE),
            "c_onesz": np.ones((128, 32), E4M3),
            "c_warm": np.ones((128, 288), E4M3),
            "c_onesb": np.ones((128, 1), ml_dtypes.bfloat16),
            "c_onesb1": np.ones((1, 128), ml_dtypes.bfloat16),
        })
    return in_maps


def kernel(**inputs):
    nc = _get_program()
    in_maps = _make_in_maps(**inputs)
    res = run_bass_kernel_spmd(nc, in_maps, list(range(8)))
    out = np.empty((B, 1, CV, HW), np.float32)
    for core in range(8):
        b, s = divmod(core, 4)
        out[b, 0, :, s * NCORE:(s + 1) * NCORE] = np.asarray(
            res.results[core]["out"], dtype=np.float32)
    return out.reshape(B, 1, CV, H, W)


if __name__ == "__main__":
    rng = np.random.default_rng(0)
    dummy = {
        "query_key": rng.standard_normal((B, CK, H, W)).astype(np.float32),
        "query_selection": rng.random((B, CK, H, W)).astype(np.float32),
        "memory_key": rng.standard_normal((B, CK, T, H, W)).astype(np.float32),
        "memory_shrinkage": rng.random((B, 1, T, H, W)).astype(np.float32),
        "msk_value": rng.standard_normal((B, 1, CV, T, H, W)).astype(np.float32),
        "uncert_prob": rng.random((B, 1, H, W)).astype(np.float32),
    }
    out = kernel(**dummy)
    print("out", out.shape, out.dtype, float(np.abs(out).mean()))


# revision 35
# speedup vs baseline: 1.0104x; 1.0104x over previous
"""MatAnyone memory-readout kernel for 8 Trainium2 NeuronCores (fp8 DoubleRow).

Math (per batch b, query pixel n, memory slot t):
  sim[t,n] = ms[t]*(-a_sq + 2ab - b_sq)[t,n]/sqrt(CK)
  aff      = softmax_t(sim);  R[c,n] = sum_t mv[c,t]*aff[t,n]
  out[c,n] = R[c,n]*p[n] + lv[c,n]*(1-p[n])

Sharding: 8 cores = 2 batches x 4 query-pixel shards (n = 576 per core).

Per-core plan (fp8 e4m3 DoubleRow matmuls; DR = K-paired contraction,
out = sum_i W[:,i].T @ X[:,i], both operands fp8, ~2x column rate):
  sim: Ki=65 padded to 128. lhsT pairs = [ms*mk^2/2 | 2*ms*mk] per channel
       plus a 65th row (2ms | ms/4) that folds in -b_sq*ms/8 (fp8 residual
       correction in the second slot). rhs pairs = [-qe/2 | qe*qk/4] plus
       (-b_sq/8 | 8*residual). One DR matmul per (t-tile, n-half) writes
       psum = 2*sim; halves live at 512-aligned offsets (psum bank-crossing
       matmul writes are broken on trn2 - everything is bank-aligned).
       ACT: E = exp(0.5*psum) -> fp8, one instr per t-tile via 3D AP.
  R:   t-tiles paired (K=256 over t): lhsT = mv pair-chunk [128,2,128],
       rhs = E-pair [128,2,288]. 4 DR matmuls per pair, accumulated over
       all 72 pairs in psum.
  Z:   odd pairs + the last few on PE (DR ones-weight, M=1, N=144 segments
       in the R banks' gap columns); remaining pairs on DVE (G2 += E-pair);
       G2's partition sums are folded into the same psum Z segments by bf16
       ones-matmuls overlapped with the loop tail. 1/(2Z) runs on ACT
       (Reciprocal, accuracy is ample for the 2e-2 gate), with 2x folded
       into p on the host.
  PSUM (8 banks): RZ tile [128,2048] = R quarters @ bank starts (cols
       (2k+hh)*512) + Z segments @ s*512+288 (4 banks); sim pool
       [128,1024] x 2 bufs (2 banks each). PE warmup matmuls trip the HAM
       clock-gate to 8/8 during the initial DMA wait.
  out = R*(2p/2Z) + lv*(1-p) in bf16; lv*(1-p) precomputed on host.
"""

import sys

for _p in ("/opt/trn_rl_repo", "/root/.axon_site/_ro/trn_rl_repo"):
    if _p not in sys.path:
        sys.path.insert(0, _p)

from contextlib import ExitStack

import numpy as np
import ml_dtypes

import concourse.bass as bass
from concourse import mybir
from concourse.bacc import Bacc
from concourse.tile import TileContext
from concourse.bass_utils import run_bass_kernel_spmd

F32 = mybir.dt.float32
BF16 = mybir.dt.bfloat16
FP8 = mybir.dt.float8e4
EXP = mybir.ActivationFunctionType.Exp
DR = mybir.MatmulPerfMode.DoubleRow
E4M3 = ml_dtypes.float8_e4m3

B, CK, CV, T, H, W = 2, 64, 256, 8, 48, 48
HW = H * W            # 2304
THW = T * HW          # 18432
NCORE = HW // 4       # 576 query pixels per core
NH = NCORE // 2       # 288 per n-half
TT = THW // 128       # 144 t-tiles
NPAIR = TT // 2       # 72 t-tile pairs
SKEW = 3              # pairs of lag between exp and readout
CHP = 8               # pairs per streamed mkw chunk
Z_PE_MOD = 10**9          # pair a accumulates Z on PE iff a % Z_PE_MOD == 0

_CACHE = {}


def _fp8(x):
    return np.clip(x, -240.0, 240.0).astype(E4M3)


def build_program():
    nc = Bacc(name="matanyone_fp8dr")

    qw_h = nc.declare_dram_parameter("qw", [128, 2 * NCORE], FP8, isOutput=False)
    mkw_h = nc.declare_dram_parameter("mkw", [128, TT * 256], FP8,
                                      isOutput=False)
    mvw_h = nc.declare_dram_parameter("mvw", [128, NPAIR * 512], FP8,
                                      isOutput=False)
    lvw2_h = nc.declare_dram_parameter("lvw2", [CV, NCORE], BF16,
                                       isOutput=False)
    p_h = nc.declare_dram_parameter("p", [1, NCORE], F32, isOutput=False)
    cz_h = nc.declare_dram_parameter("c_onesz", [128, 32], FP8, isOutput=False)
    cw_h = nc.declare_dram_parameter("c_warm", [128, 288], FP8, isOutput=False)
    cb_h = nc.declare_dram_parameter("c_onesb", [128, 1], BF16, isOutput=False)
    cb1_h = nc.declare_dram_parameter("c_onesb1", [1, 128], BF16, isOutput=False)
    out_h = nc.declare_dram_parameter("out", [CV, NCORE], BF16, isOutput=True)


    with TileContext(nc) as tc, ExitStack() as ctx:
        persist = ctx.enter_context(tc.tile_pool(name="persist", bufs=1))
        ps_rz0 = ctx.enter_context(tc.tile_pool(name="psrz", bufs=1,
                                                space="PSUM"))
        mvpool = ctx.enter_context(tc.tile_pool(name="mv", bufs=1))
        m2pool = ctx.enter_context(tc.tile_pool(name="m2", bufs=2))
        epool = ctx.enter_context(tc.tile_pool(name="E", bufs=SKEW + 2))
        ps_sim = ctx.enter_context(tc.tile_pool(name="pssim", bufs=2,
                                                space="PSUM"))
        fin = ctx.enter_context(tc.tile_pool(name="fin", bufs=1))

        # RZ: R quarters (k,hh) @ (2k+hh)*512 + Z segs @ s*512+288
        rz = ps_rz0.tile([128, 2048], F32, tag="rz")

        # ---- persistent inputs / constants --------------------------------
        onesz = persist.tile([128, 32], FP8, tag="onesz")
        nc.sync.dma_start(out=onesz[:], in_=cz_h[:])
        wrm = persist.tile([128, 288], FP8, tag="wrm")
        nc.sync.dma_start(out=wrm[:], in_=cw_h[:])
        qw = persist.tile([128, 2 * NCORE], FP8, tag="qw")
        for _o in (0, 576):   # halves used by hh=0 first
            nc.sync.dma_start(out=qw[:, _o:_o + NH], in_=qw_h[:, _o:_o + NH])
        for _o in (288, 864):
            nc.sync.dma_start(out=qw[:, _o:_o + NH], in_=qw_h[:, _o:_o + NH])
        onesb = persist.tile([128, 1], BF16, tag="onesb")
        nc.sync.dma_start(out=onesb[:], in_=cb_h[:])
        onesb1 = persist.tile([1, 128], BF16, tag="onesb1")
        nc.sync.dma_start(out=onesb1[:], in_=cb1_h[:])
        g2 = persist.tile([128, 2 * NCORE], F32, tag="g2")

        qw3 = qw.rearrange("p (i n) -> p i n", i=2)
        onesz3 = onesz.rearrange("p (i m) -> p i m", i=2)

        # PE warmup: real-sized matmuls into the Z-gap region trip the HAM
        # to 8/8 during the initial DMA wait; the first real Z matmul
        # (start=True) resets the region.
        wrm3 = wrm.rearrange("p (i n) -> p i n", i=2)
        for _w in range(40):
            nc.tensor.matmul(rz[0:1, 288:432], onesz3[:, :, 0:1], wrm3[:],
                             start=True, stop=True, perf_mode=DR)

        # resident mv weights, DMA'd in chunks inside the loop
        mvw = mvpool.tile([128, NPAIR * 512], FP8, tag="mvw")

        e_tiles = {}
        mkc = None

        def pair_front(a):
            nonlocal mkc
            if a % CHP == 0:
                g = a // CHP
                mkc = m2pool.tile([128, CHP * 512], FP8, tag="mkc")
                if g == 0:
                    for q in range(CHP):
                        nc.sync.dma_start(
                            out=mkc[:, q * 512:(q + 1) * 512],
                            in_=mkw_h[:, q * 512:(q + 1) * 512])
                    for q in range(CHP):
                        nc.scalar.dma_start(
                            out=mvw[:, q * 512:(q + 1) * 512],
                            in_=mvw_h[:, q * 512:(q + 1) * 512])
                else:
                    nc.sync.dma_start(
                        out=mkc[:],
                        in_=mkw_h[:, g * CHP * 512:(g + 1) * CHP * 512])
                    nc.sync.dma_start(
                        out=mvw[:, g * CHP * 512:(g + 1) * CHP * 512],
                        in_=mvw_h[:, g * CHP * 512:(g + 1) * CHP * 512])
            e = epool.tile([128, 2 * NCORE], FP8, tag="E")

            def sim_tile(j):
                wsl = mkc[:, (a % CHP) * 512 + j * 256:(a % CHP) * 512
                          + (j + 1) * 256]
                w3 = wsl.rearrange("p (i m) -> p i m", i=2)
                sim = ps_sim.tile([128, 1024], F32, tag="sim")
                for hh in (0, 1):
                    nc.tensor.matmul(sim[:, hh * 512:hh * 512 + NH], w3,
                                     qw3[:, :, hh * NH:(hh + 1) * NH],
                                     start=True, stop=True, perf_mode=DR)
                sim3 = sim.rearrange("p (i n) -> p i n", i=2)[:, :, 0:NH]
                e2 = e[:, j * NCORE:(j + 1) * NCORE].rearrange(
                    "p (i n) -> p i n", i=2)
                nc.scalar.activation(e2, sim3, EXP, scale=0.5)

            e_tiles[a] = (e, sim_tile)

        def pair_back_k(a, k):
            e = e_tiles[a][0]
            e3 = e.rearrange("p (i n) -> p i n", i=2)
            st, sp = (a == 0), (a == NPAIR - 1)
            wsl = mvw[:, a * 512 + k * 256:a * 512 + (k + 1) * 256]
            w3 = wsl.rearrange("p (i m) -> p i m", i=2)
            for hh in (0, 1):
                q = (2 * k + hh) * 512
                nc.tensor.matmul(
                    rz[:, q:q + NH],
                    w3, e3[:, :, hh * NH:(hh + 1) * NH],
                    start=st, stop=sp, perf_mode=DR)

        def pair_back_z(a):
            e = e_tiles.pop(a)[0]
            e3 = e.rearrange("p (i n) -> p i n", i=2)
            if a % 2 == 1 or a >= NPAIR - 5:
                for s in range(4):
                    nc.tensor.matmul(
                        rz[0:1, s * 512 + 288:s * 512 + 432],
                        onesz3[:, :, 0:1], e3[:, :, s * 144:(s + 1) * 144],
                        start=(a == 1), stop=False, perf_mode=DR)
            elif a == 0:
                nc.vector.tensor_copy(g2[:], e[:])
            else:
                nc.vector.tensor_add(g2[:], g2[:], e[:])
            if a == NPAIR - 5:
                # all remaining Z goes to PE; G2 is final -> fold it now so
                # the cast+fold matmuls overlap the tail of the main loop
                gb = fin.tile([128, 2 * NCORE], BF16, tag="gb")
                nc.vector.tensor_copy(gb[:], g2[:])
                for s in range(4):
                    for i in (0, 1):
                        nc.tensor.matmul(
                            rz[0:1, s * 512 + 288:s * 512 + 432], onesb[:],
                            gb[:, i * NCORE + s * 144:i * NCORE
                               + (s + 1) * 144],
                            start=False, stop=(i == 1))

        for a in range(NPAIR + SKEW):
            if a < NPAIR:
                pair_front(a)
                e_tiles[a][1](0)
                e_tiles[a][1](1)
            if a >= SKEW:
                pair_back_k(a - SKEW, 0)
                pair_back_k(a - SKEW, 1)
                pair_back_z(a - SKEW)

        # ---- finalize ------------------------------------------------------
        p_sb = persist.tile([1, NCORE], F32, tag="p")
        nc.sync.dma_start(out=p_sb[:], in_=p_h[:])
        lvw2 = []
        for k in (0, 1):
            t = persist.tile([128, NCORE], BF16, tag=f"lvw2{k}")
            nc.sync.dma_start(out=t[:], in_=lvw2_h[k * 128:(k + 1) * 128, :])
            lvw2.append(t)
        # 1/(2Z) on the scalar engine straight from psum (p has 2x folded in)
        rzv = fin.tile([1, NCORE], F32, tag="rzv")
        rz4 = rz.rearrange("p (s c) -> p s c", c=512)[0:1, :, 288:432]
        eng = nc.scalar
        eng.add_instruction(mybir.InstActivation(
            name=nc.get_next_instruction_name(),
            func=mybir.ActivationFunctionType.Reciprocal,
            ins=[eng.lower_ap(rz4),
                 mybir.ImmediateValue(dtype=mybir.dt.float32, value=0.0),
                 mybir.ImmediateValue(dtype=mybir.dt.float32, value=2.0),
                 mybir.ImmediateValue(dtype=mybir.dt.float32, value=0.0)],
            outs=[eng.lower_ap(rzv.rearrange("p (s n) -> p s n", s=4))]))
        w1 = fin.tile([1, NCORE], BF16, tag="w1")
        nc.vector.tensor_mul(w1[:], rzv[:], p_sb[:])          # 2p / 2Z

        w1s = fin.tile([128, NCORE], F32, tag="w1s")
        wt = ps_sim.tile([128, 1024], F32, tag="sim")
        for hh in (0, 1):
            nc.tensor.matmul(wt[:, hh * 512:hh * 512 + NH], onesb1[:],
                             w1[:, hh * NH:(hh + 1) * NH],
                             start=True, stop=True)
        wt3 = wt.rearrange("p (i n) -> p i n", i=2)[:, :, 0:NH]
        nc.vector.tensor_copy(w1s.rearrange("p (i n) -> p i n", i=2), wt3)

        for k in (0, 1):
            o = fin.tile([128, NCORE], BF16, tag="o", bufs=2)
            rk = rz[:, k * 1024:k * 1024 + 1024].rearrange(
                "p (i n) -> p i n", i=2)[:, :, 0:NH]
            nc.vector.tensor_mul(o.rearrange("p (i n) -> p i n", i=2), rk,
                                 w1s.rearrange("p (i n) -> p i n", i=2))
            nc.vector.tensor_add(o[:], o[:], lvw2[k][:])
            nc.sync.dma_start(out=out_h[k * 128:(k + 1) * 128, 0:NH],
                              in_=o[:, 0:NH])
            nc.scalar.dma_start(out=out_h[k * 128:(k + 1) * 128, NH:NCORE],
                                in_=o[:, NH:NCORE])

    nc.finalize()
    return nc


def _get_program():
    if "nc" not in _CACHE:
        _CACHE["nc"] = build_program()
    return _CACHE["nc"]


def _make_in_maps(query_key, query_selection, memory_key, memory_shrinkage,
                  msk_value, uncert_prob):
    qk = np.asarray(query_key, np.float32).reshape(B, CK, HW)
    qe = np.asarray(query_selection, np.float32).reshape(B, CK, HW)
    mk = np.asarray(memory_key, np.float32).reshape(B, CK, THW)
    ms = np.asarray(memory_shrinkage, np.float32).reshape(B, THW)
    mv = np.asarray(msk_value, np.float32).reshape(B, CV, THW)
    lv = np.asarray(msk_value, np.float32).reshape(B, CV, T, HW)[:, :, T - 1, :]
    p = np.asarray(uncert_prob, np.float32).reshape(B, HW)

    # per-batch sim weights: [65, TT, 2, 128]
    mkw_b = []
    mvw_b = []
    for b in range(B):
        mk3 = mk[b].reshape(CK, TT, 128)               # [c, tau, m]
        ms3 = ms[b].reshape(TT, 128)                   # [tau, m]
        mkw = np.zeros((128, TT, 2, 128), np.float32)
        mkw[:CK, :, 0, :] = ms3[None] * mk3 * mk3 * 0.5
        mkw[:CK, :, 1, :] = 2.0 * ms3[None] * mk3
        mkw[64, :, 0, :] = 2.0 * ms3
        mkw[64, :, 1, :] = 0.25 * ms3
        mkw_b.append(_fp8(mkw).reshape(128, TT * 256))
        # mv DR weights: [p, a, k, i, m] = mv[k*128+m, (2a+i)*128+p]
        tmp = mv[b].reshape(2, 128, NPAIR, 2, 128)     # [k, m, a, i, p]
        mvw = tmp.transpose(4, 2, 0, 3, 1).reshape(128, NPAIR * 512)
        mvw_b.append(_fp8(mvw))

    in_maps = []
    for core in range(8):
        b, s = divmod(core, 4)
        sl = slice(s * NCORE, (s + 1) * NCORE)
        qks, qes = qk[b, :, sl], qe[b, :, sl]
        bsq = np.einsum("cn,cn->n", qes, qks * qks)    # [576]
        qwf = np.zeros((128, 2, NCORE), np.float32)
        qwf[:CK, 0] = -0.5 * qes
        qwf[:CK, 1] = 0.25 * qes * qks
        b0 = _fp8(-bsq / 8.0)
        qwf[64, 0] = b0.astype(np.float32)
        qwf[64, 1] = 8.0 * (-bsq / 8.0 - b0.astype(np.float32))
        ps = p[b, sl]
        in_maps.append({
            "qw": _fp8(qwf).reshape(128, 2 * NCORE),
            "mkw": mkw_b[b],
            "mvw": mvw_b[b],
            "lvw2": np.ascontiguousarray(
                lv[b, :, sl] * (1.0 - ps)[None, :]).astype(ml_dtypes.bfloat16),
            "p": np.ascontiguousarray(ps).reshape(1, NCORE),
            "c_onesz": np.ones((128, 32), E4M3),
            "c_warm": np.ones((128, 288), E4M3),
            "c_onesb": np.ones((128, 1), ml_dtypes.bfloat16),
            "c_onesb1": np.ones((1, 128), ml_dtypes.bfloat16),
        })
    return in_maps


def kernel(**inputs):
    nc = _get_program()
    in_maps = _make_in_maps(**inputs)
    res = run_bass_kernel_spmd(nc, in_maps, list(range(8)))
    out = np.empty((B, 1, CV, HW), np.float32)
    for core in range(8):
        b, s = divmod(core, 4)
        out[b, 0, :, s * NCORE:(s + 1) * NCORE] = np.asarray(
            res.results[core]["out"], dtype=np.float32)
    return out.reshape(B, 1, CV, H, W)


if __name__ == "__main__":
    rng = np.random.default_rng(0)
    dummy = {
        "query_key": rng.standard_normal((B, CK, H, W)).astype(np.float32),
        "query_selection": rng.random((B, CK, H, W)).astype(np.float32),
        "memory_key": rng.standard_normal((B, CK, T, H, W)).astype(np.float32),
        "memory_shrinkage": rng.random((B, 1, T, H, W)).astype(np.float32),
        "msk_value": rng.standard_normal((B, 1, CV, T, H, W)).astype(np.float32),
        "uncert_prob": rng.random((B, 1, H, W)).astype(np.float32),
    }
    out = kernel(**dummy)
    print("out", out.shape, out.dtype, float(np.abs(out).mean()))
